# revision 8
# baseline (speedup 1.0000x reference)
"""Trainium2 Bass kernel for AdaptiveCLPLLoss.

Reference computation (B=512, C=100000, HEAD=2000, K=10, S=100):
    logits  [B, C] f32, candidates [B, K] i64, sampled_idx [S] i64
    y_mask  = binarized scatter of valid candidates          [B, C]
    term1   = softplus(-avg_cand)        avg over distinct candidate logits
    term2   = sum over head cols of softplus(logits) * (1 - y_mask)
    term3   = sum over sampled tail cols of softplus(logits) * not_cand * 980
    loss    = mean over batch of (term1 + term2 + term3)

Only ~2110 of the 100000 columns are ever read per row.  Sharding:
data-parallel over batch (64 rows per core, 8 cores); each core's kernel
reads the head block plus two indirect-DMA gathers (candidate positions,
sampled tail columns) out of its DRAM-resident logits shard, computes
per-row partial losses, and the host mean-reduces the 512 partials.

softplus(x) = Ln(1*exp(x) + 1) on the Scalar engine: both funcs live in
the natural_log_exp_and_others ACT table set (one table load), and the
"+1" rides the activation's free affine bias.  Row sums ride accum_out.
"""

import sys

if "/opt/trn_rl_repo" not in sys.path:
    sys.path.insert(0, "/opt/trn_rl_repo")

import numpy as np

B, C, HEAD, K, S = 512, 100000, 2000, 10, 100
NCORES = 8
RB = B // NCORES            # rows per core
TAIL = C - HEAD
SCALE3 = float(TAIL) / S    # 980.0
HALF = HEAD // 2            # head laid out as [128, HALF] on chip

_BUILT = None


def _legalize_waits(nc):
    """Split >cap sync waits onto preceding NoOps (walrus codegen accepts at
    most 1 wait per instruction, 2 on EventSemaphore; Tile attaches more)."""
    from concourse import mybir

    cnt = 0
    for bfn in nc.m.functions:
        for blk in bfn.blocks:
            out = []
            changed = False
            for inst in blk.instructions:
                si = inst.sync_info
                waits = list(si.on_wait) if si is not None and si.on_wait else []
                cap = 2 if isinstance(inst, mybir.InstEventSemaphore) else 1
                if len(waits) > cap:
                    changed = True
                    keep = waits[-cap:]
                    for w in waits[:-cap]:
                        cnt += 1
                        out.append(mybir.InstNoOp(
                            name=f"WSPLIT-{cnt}",
                            engine=inst.engine,
                            sync_info=mybir.SyncInfo(on_wait=[w], on_update=[]),
                            bass_nofuse=True,
                        ))
                    inst.sync_info = mybir.SyncInfo(
                        on_wait=keep,
                        on_update=list(si.on_update) if si.on_update else [],
                    )
                out.append(inst)
            if changed:
                blk.instructions = out
    return nc


def _build():
    from concourse import bass, mybir, tile

    f32 = mybir.dt.float32
    i32 = mybir.dt.int32
    F = mybir.ActivationFunctionType
    A = mybir.AluOpType

    nc = bass.Bass()
    logits = nc.declare_dram_parameter("logits", [RB, C], f32, isOutput=False)
    cand_off = nc.declare_dram_parameter("cand_off", [RB, K], i32, isOutput=False)
    samp_off = nc.declare_dram_parameter("samp_off", [RB, S], i32, isOutput=False)
    w1 = nc.declare_dram_parameter("w1", [RB, K], f32, isOutput=False)
    w2 = nc.declare_dram_parameter("w2", [RB, K], f32, isOutput=False)
    m3 = nc.declare_dram_parameter("m3", [RB, S], f32, isOutput=False)
    out = nc.dram_tensor("out", [RB, 1], f32, kind="ExternalOutput")

    with tile.TileContext(nc) as tc:
        with tc.tile_pool(name="p", bufs=1) as pool:
            # --- head block: [RB, HEAD] ----------------------------------
            head_t = pool.tile([RB, HEAD], f32)
            nc.sync.dma_start(out=head_t[:], in_=logits[:, 0:HEAD])
            exp_t = pool.tile([RB, HEAD], f32)
            nc.scalar.activation(exp_t[:], head_t[:], F.Exp)
            sp_t = pool.tile([RB, HEAD], f32)
            head_acc = pool.tile([RB, 1], f32)
            nc.scalar.activation(
                sp_t[:], exp_t[:], F.Ln, bias=1.0, accum_out=head_acc[:]
            )

            # --- small aux inputs ----------------------------------------
            co_t = pool.tile([RB, K], i32)
            nc.sync.dma_start(out=co_t[:], in_=cand_off[:])
            so_t = pool.tile([RB, S], i32)
            nc.sync.dma_start(out=so_t[:], in_=samp_off[:])
            w1_t = pool.tile([RB, K], f32)
            nc.sync.dma_start(out=w1_t[:], in_=w1[:])
            w2_t = pool.tile([RB, K], f32)
            nc.sync.dma_start(out=w2_t[:], in_=w2[:])
            m3_t = pool.tile([RB, S], f32)
            nc.sync.dma_start(out=m3_t[:], in_=m3[:])

            # --- gathers out of the DRAM logits shard --------------------
            cand_t = pool.tile([RB, K], f32)
            nc.gpsimd.indirect_dma_start(
                out=cand_t[:],
                out_offset=None,
                in_=logits[:],
                in_offset=bass.IndirectOffsetOnAxis(ap=co_t[:], axis=1),
            )
            samp_t = pool.tile([RB, S], f32)
            nc.gpsimd.indirect_dma_start(
                out=samp_t[:],
                out_offset=None,
                in_=logits[:],
                in_offset=bass.IndirectOffsetOnAxis(ap=so_t[:], axis=1),
            )

            # --- term3: 980 * sum softplus(sampled) * not_cand -----------
            sexp = pool.tile([RB, S], f32)
            nc.scalar.activation(sexp[:], samp_t[:], F.Exp)
            ssp = pool.tile([RB, S], f32)
            nc.scalar.activation(ssp[:], sexp[:], F.Ln, bias=1.0)
            t3p = pool.tile([RB, S], f32)
            t3_row = pool.tile([RB, 1], f32)
            nc.vector.scalar_tensor_tensor(
                out=t3p[:], in0=ssp[:], scalar=1.0, in1=m3_t[:],
                op0=A.mult, op1=A.mult, accum_out=t3_row[:],
            )

            # --- candidate terms -----------------------------------------
            cexp = pool.tile([RB, K], f32)
            nc.scalar.activation(cexp[:], cand_t[:], F.Exp)
            csp = pool.tile([RB, K], f32)
            nc.scalar.activation(csp[:], cexp[:], F.Ln, bias=1.0)
            c2p = pool.tile([RB, K], f32)
            c2_row = pool.tile([RB, 1], f32)
            nc.vector.scalar_tensor_tensor(
                out=c2p[:], in0=csp[:], scalar=1.0, in1=w2_t[:],
                op0=A.mult, op1=A.mult, accum_out=c2_row[:],
            )
            avgp = pool.tile([RB, K], f32)
            avg_row = pool.tile([RB, 1], f32)
            nc.vector.scalar_tensor_tensor(
                out=avgp[:], in0=cand_t[:], scalar=1.0, in1=w1_t[:],
                op0=A.mult, op1=A.mult, accum_out=avg_row[:],
            )
            t1e = pool.tile([RB, 1], f32)
            nc.scalar.activation(t1e[:], avg_row[:], F.Exp, scale=-1.0)
            t1_row = pool.tile([RB, 1], f32)
            nc.scalar.activation(t1_row[:], t1e[:], F.Ln, bias=1.0)

            # --- combine: t1 + (head - c2) + t3 --------------------------
            s1 = pool.tile([RB, 1], f32)
            nc.vector.tensor_tensor(out=s1[:], in0=head_acc[:], in1=c2_row[:],
                                    op=A.subtract)
            s2 = pool.tile([RB, 1], f32)
            nc.vector.tensor_tensor(out=s2[:], in0=s1[:], in1=t1_row[:],
                                    op=A.add)
            tot = pool.tile([RB, 1], f32)
            nc.vector.tensor_tensor(out=tot[:], in0=s2[:], in1=t3_row[:],
                                    op=A.add)
            nc.sync.dma_start(out=out[:], in_=tot[:])

    _legalize_waits(nc)
    return nc


def _get_built():
    global _BUILT
    if _BUILT is None:
        _BUILT = _build()
    return _BUILT


def _host_prep(logits, candidates, sampled_idx):
    """Index-only host prep: dedup/weight masks + flat gather offsets."""
    cand = np.asarray(candidates)
    samp = np.asarray(sampled_idx).reshape(-1)
    valid = cand >= 0                                        # [B, K]

    # first-occurrence mask over valid candidates (set semantics)
    W = np.zeros((B, K), np.float32)
    for k in range(K):
        dup = np.zeros(B, bool)
        for j in range(k):
            dup |= valid[:, j] & (cand[:, j] == cand[:, k])
        W[:, k] = (valid[:, k] & ~dup).astype(np.float32)

    ycard = np.maximum(W.sum(axis=1), 1.0).astype(np.float32)   # [B]
    w1 = (W / ycard[:, None]).astype(np.float32)                # [B, K]
    w2 = (W * (cand < HEAD)).astype(np.float32)                 # [B, K]

    g = (HEAD + samp).astype(np.int64)                          # [S]
    is_cand = (valid[:, :, None] & (cand[:, :, None] == g[None, None, :])).any(
        axis=1
    )                                                           # [B, S]
    m3 = (SCALE3 * (~is_cand)).astype(np.float32)               # [B, S]

    cand_pos = np.where(valid, cand, 0).astype(np.int64)        # [B, K]
    row = np.arange(RB, dtype=np.int64)[:, None]
    samp_off = (row * C + g[None, :]).astype(np.int32)          # [RB, S]
    return w1, w2, m3, cand_pos, samp_off, row


def kernel(logits, candidates, sampled_idx):
    from concourse.bass_utils import run_bass_kernel_spmd

    logits = np.ascontiguousarray(np.asarray(logits, dtype=np.float32))
    w1, w2, m3, cand_pos, samp_off, row = _host_prep(
        logits, candidates, sampled_idx
    )

    in_maps = []
    for i in range(NCORES):
        sl = slice(i * RB, (i + 1) * RB)
        in_maps.append({
            "logits": logits[sl],
            "cand_off": (row * C + cand_pos[sl]).astype(np.int32),
            "samp_off": samp_off,
            "w1": w1[sl],
            "w2": w2[sl],
            "m3": m3[sl],
        })

    nc = _get_built()
    res = run_bass_kernel_spmd(nc, in_maps, core_ids=list(range(NCORES)))
    per_row = np.concatenate([res.results[i]["out"].reshape(-1)
                              for i in range(NCORES)])
    return np.float32(per_row.sum(dtype=np.float64) / B)
